# revision 33
# baseline (speedup 1.0000x reference)
"""Causal multi-head attention (nn_Attention_87840671138123) on 8 trn2 NeuronCores.

Problem (B=2, S=2048, D=1024, H=16 heads, E=64 head_dim), fp32:
    Q = einsum('bsd,hde->bhse', q, W_q)   (same for K, V)
    scores = Q @ K^T / sqrt(D), causal mask, softmax
    attn = probs @ V  -> [B, S, D] (head-major concat)
    out = attn @ W_o.T

Sharding: core = 4*b + quad. Each core handles batch b and a quad of 4 heads
(heads 4*quad .. 4*quad+3). It computes a partial output
    out_part = attn_quad @ W_o.T[quad rows, :]   [S, D]
and the host sums the 4 partials per batch (the "all-reduce" of the output
projection done host-side at gather time).

Device layout choices (per core):
 - Host passes xT = x[b].T  [D, S] so the d-contraction sits on partitions.
 - Projections produce QT/KT in "transposed" layout [head-pair x 64, S]
   (head h2 of a pair occupies partitions 64*h2..64*h2+63), and V in natural
   [t, e] layout augmented with a ones-column (V_aug [t, 65]) so the
   attn matmul also accumulates the softmax denominator as row 64.
 - scoresT[t, s] = (KT chunk).T @ QT  -> exp on ACT (scale 1/32 folded in)
   -> causal handled by (a) skipping fully-masked blocks, (b) shrinking the
   moving dim to the valid s-range for diagonal blocks, (c) one [128,128]
   triangular mask multiply for the diagonal 128-col strip.
 - attnT_aug[65, s] += V_aug.T @ expT accumulated over t chunks in PSUM.
 - Normalize: denom row -> reciprocal -> partition_broadcast -> multiply.
 - Output projection: out[s, :] = sum_g (attnT chunk).T @ W_o.T slice.

Numerics: the Q/K path (inputs, weights, DoubleRow projections, stored
QT/KT, scores matmul) runs in fp8e4m3 — score errors are absolute-small
and only perturb softmax weights (~5e-3 relative on the output, vs the
2e-2 gate). The V path (V projection, attn*V) stays fp16 and the output
projection (attnG/WoT) bf16: value errors there pass straight through.

Performance model (measured): the NC clock is power/activity-throttled
(type-1 HAM throttle, half clock); MAC switching energy drives it, so
every padding element is zero and the hot paths use the narrowest dtypes
accuracy allows. fp8 scores + zero padding took the throttle duty from
~30% to ~14% of the run. Q/K projections use fp8 DoubleRow (K=256/matmul,
~2x). The epilogue normalize uses cross-base-partition DVE ops (PSUM row
64 -> partition 0, rows 0..63 -> 64..127) instead of SBUF-SBUF DMAs.
Measured on 8 cores: 229 us (staged baseline) -> 175.3 us.
"""

import ml_dtypes
import numpy as np

import concourse.bass as bass
import concourse.tile as tile
from concourse import bacc, mybir
from concourse.bass_utils import run_bass_kernel_spmd

B, S, D, H, E = 2, 2048, 1024, 16, 64
P = 128
NCORES = 8
SJ = 512            # s-tile width
NJ = S // SJ        # 4 s-tiles
ND = D // P         # 8 d-chunks
NT = S // P         # 16 t-chunks
f32 = mybir.dt.float32
f32r = mybir.dt.float32r
bf16 = mybir.dt.bfloat16
fp16 = mybir.dt.float16
EXP = mybir.ActivationFunctionType.Exp
MULT = mybir.AluOpType.mult

QK_DT = mybir.dt.float8e4   # q/k inputs, Wq/Wk, DoubleRow projection matmuls
SC_DT = mybir.dt.float8e4   # dtype of stored QT/KT + scores matmul (fp8e4m3)
V_DT = fp16         # dtype of v input, Wv, V_aug, expT
O_DT = bf16         # dtype of attnG / WoT (output projection matmul)
DR = mybir.MatmulPerfMode.DoubleRow

_NP_OF = {bf16: ml_dtypes.bfloat16, fp16: np.float16, f32r: np.float32,
          f32: np.float32, mybir.dt.float8e4: ml_dtypes.float8_e4m3}

_NC_CACHE = []


def _patch_ldw_opt():
    """Enable walrus LDWEIGHTS optimization (fast weight load).

    bass_utils.bir_verify_and_optimise hardcodes --enable-ldw-opt=false;
    wrap it to rewrite the flag. With ldw-opt on, walrus rejects standalone
    InstLdweights (CoreV3GenImpl.cpp:694) and instead generates the weight
    load itself from self-loading Matmults — so also rewrite bir.json to
    re-fuse each Ldweights+Matmult pair (bass tile_legalize splits them):
    drop the Ldweights, set ldweights=true on the Matmult, and merge the
    semaphore waits. A matmul carries one wait; extras (36 of 640 pairs)
    are parked on an inserted PE EventSemaphore just before the matmul.
    Verified numerically by the test harness.
    """
    import json as _json
    import os as _os

    from concourse import bass_utils as _bu
    if getattr(_bu, "_ldw_patched", False):
        return
    _orig_run = _bu.run_command

    def _rewrite_bir(path):
        with open(path) as f:
            d = _json.load(f)
        nfix = 0
        for fn in d.get("functions", []):
            for blk in fn.get("blocks", []):
                out = []
                pending = None
                for ins in blk["instructions"]:
                    eng = ins.get("engine")
                    op = ins.get("opcode")
                    if eng == "PE" and op == "Ldweights":
                        assert pending is None, "unpaired Ldweights"
                        pending = ins
                        continue
                    if eng == "PE" and op == "Matmult" and pending is not None:
                        waits = list(
                            (pending.get("sync_info") or {}).get("on_wait") or [])
                        si = ins.get("sync_info") or {
                            "on_update": [], "on_wait": []}
                        waits += list(si.get("on_wait") or [])
                        for w in waits[:-1]:
                            nfix += 1
                            out.append({
                                "debug": ins.get("debug", 0),
                                "engine": "PE",
                                "ins": [],
                                "name": f"I-ldwfix-{nfix}",
                                "opcode": "EventSemaphore",
                                "outs": [],
                                "sync_info": {"on_update": [], "on_wait": [w]},
                            })
                        si["on_wait"] = waits[-1:]
                        ins["sync_info"] = si
                        ins["ldweights"] = True
                        pending = None
                    out.append(ins)
                assert pending is None, "trailing Ldweights"
                blk["instructions"] = out
        with open(path, "w") as f:
            _json.dump(d, f)

    def _run(argv, **kw):
        argv = list(argv)
        if "--enable-ldw-opt=false" in argv:
            argv = ["--enable-ldw-opt=true" if a == "--enable-ldw-opt=false"
                    else a for a in argv]
            try:
                birp = argv[argv.index("-i") + 1]
            except ValueError:
                birp = None
            if birp is not None:
                cwd = kw.get("cwd")
                if cwd and not _os.path.isabs(birp):
                    birp = _os.path.join(cwd, birp)
                _rewrite_bir(birp)
        return _orig_run(argv, **kw)

    _bu.run_command = _run
    _bu._ldw_patched = True


def _build():
    _patch_ldw_opt()
    nc = bacc.Bacc("TRN2", target_bir_lowering=False, debug=False)

    qT_d = nc.dram_tensor("qT", [D, S], QK_DT, kind="ExternalInput")
    kT_d = nc.dram_tensor("kT", [D, S], QK_DT, kind="ExternalInput")
    vT_d = nc.dram_tensor("vT", [D, S], V_DT, kind="ExternalInput")
    wq_d = nc.dram_tensor("wq", [D, 4 * E], QK_DT, kind="ExternalInput")
    wk_d = nc.dram_tensor("wk", [D, 4 * E], QK_DT, kind="ExternalInput")
    wv_d = nc.dram_tensor("wv", [D, 4 * E], V_DT, kind="ExternalInput")
    wot_d = nc.dram_tensor("wot", [4 * E, D], O_DT, kind="ExternalInput")
    tri_d = nc.dram_tensor("tri", [P, P], V_DT, kind="ExternalInput")
    out_d = nc.dram_tensor("out", [S, D], f32, kind="ExternalOutput")

    with tile.TileContext(nc) as tc:
        with (
            tc.tile_pool(name="pers", bufs=1) as pers,
            tc.tile_pool(name="xt", bufs=4) as xt_pool,
            tc.tile_pool(name="ex", bufs=6) as ex_pool,
            tc.tile_pool(name="sm", bufs=6) as sm_pool,
            tc.tile_pool(name="ot", bufs=2) as ot_pool,
            tc.tile_pool(name="pj", bufs=2, space="PSUM") as pj_pool,
            tc.tile_pool(name="sc", bufs=4, space="PSUM") as sc_pool,
            tc.tile_pool(name="at", bufs=2, space="PSUM") as at_pool,
        ):
            # ---- persistent weights / constants ----
            wq_sb = pers.tile([P, ND, 4 * E], QK_DT, name="wq_sb")
            wk_sb = pers.tile([P, ND, 4 * E], QK_DT, name="wk_sb")
            wv_sb = pers.tile([P, ND, 4 * E], V_DT, name="wv_sb")
            nc.sync.dma_start(wq_sb[:], wq_d.ap().rearrange("(o p) m -> p o m", p=P))
            nc.sync.dma_start(wk_sb[:], wk_d.ap().rearrange("(o p) m -> p o m", p=P))
            nc.sync.dma_start(wv_sb[:], wv_d.ap().rearrange("(o p) m -> p o m", p=P))
            wot_sb = pers.tile([P, 2, D], O_DT, name="wot_sb")
            nc.sync.dma_start(wot_sb[:], wot_d.ap().rearrange("(g p) n -> p g n", p=P))
            tri_sb = pers.tile([P, P], V_DT, name="tri_sb")
            nc.sync.dma_start(tri_sb[:], tri_d.ap())

            # ---- persistent activations ----
            # The NC clock is power-throttled (type-1 activity throttle,
            # ~30% of the run at half clock in the bf16 baseline), so the
            # Q/K score path runs in fp8e4m3 (quantized after a bf16
            # projection) and every padding element is ZERO: zero weights
            # don't switch the PE multipliers, and MAC switching energy is
            # what drives the throttle. Score quantization only perturbs
            # softmax weights (~1% relative), not the value path.
            QT = [pers.tile([P, S], SC_DT, name=f"QT{g}") for g in range(2)]
            # Per-head KT zero-padded to 128 partitions: rows 0..63 hold the
            # head's K^T, rows 64..127 are zeros, so the K=128 scores matmul
            # nulls out the other head's Q rows in the shared QT rhs.
            KTH = [[pers.tile([P, S], SC_DT, name=f"KT{g}{h2}") for h2 in range(2)]
                   for g in range(2)]
            # V_aug padded to 128 cols: [64 V | ones | 63 zeros] so the attn
            # matmul loads all 128 PE columns (M=128).
            V = [pers.tile([P, NT, 2, P], V_DT, name=f"V{g}") for g in range(2)]
            attnG = [pers.tile([P, S], O_DT, name=f"attnG{g}") for g in range(2)]
            for g in range(2):
                nc.vector.memset(KTH[g][0][E:2 * E, :], 0.0)
                nc.vector.memset(KTH[g][1][0:E, :], 0.0)
                # only cols 0..E are ever loaded (65-col stationary)
                nc.vector.memset(V[g][:, :, :, E:E + 1], 1.0)

            def emit_wo(j):
                # output projection for s-tile j: out[s, :] = attnG.T @ WoT
                for u in range(SJ // P):
                    si = 4 * j + u
                    ot = ot_pool.tile([P, D], f32, tag="ot", name=f"ot{si}")
                    # g outer so both no-halves reuse the same stationary
                    # attnG block back-to-back; walrus dedups the repeat
                    # LDWEIGHTS (2 loads per si instead of 4)
                    pos = [pj_pool.tile([P, SJ], f32, tag="pj",
                                        name=f"po{si}{no}") for no in range(2)]
                    for g in range(2):
                        for no in range(2):
                            nc.tensor.matmul(
                                pos[no][:], attnG[g][:, bass.ts(si, P)],
                                wot_sb[:, g, bass.ts(no, SJ)],
                                start=(g == 0), stop=(g == 1))
                    for no in range(2):
                        nc.vector.tensor_copy(
                            ot[:, bass.ts(no, SJ)], pos[no][:])
                    nc.sync.dma_start(out_d.ap()[bass.ts(si, P), :], ot[:])

            # ---- fused per-s-tile pipeline: projections -> attention -> output ----
            for j in range(NJ):
                js = slice(j * SJ, (j + 1) * SJ)
                # Q/K projections in fp8 DoubleRow: each matmul contracts
                # K=256 (two 128-row k-tiles packed 2/cell), halving the PE
                # row count of the Q/K projections and the MAC energy.
                xq = xt_pool.tile([P, ND, SJ], QK_DT, tag="xtq", name=f"xq{j}")
                nc.sync.dma_start(
                    xq[:], qT_d.ap().rearrange("(o p) s -> p o s", p=P)[:, :, js])
                for g in range(2):
                    pq = pj_pool.tile([P, SJ], f32, tag="pj", name=f"pq{j}{g}")
                    for c in range(ND // 2):
                        nc.tensor.matmul(
                            pq[:], wq_sb[:, 2 * c:2 * c + 2, bass.ts(g, P)],
                            xq[:, 2 * c:2 * c + 2, :],
                            start=(c == 0), stop=(c == ND // 2 - 1),
                            perf_mode=DR)
                    nc.vector.tensor_copy(QT[g][:, js], pq[:])

                xk = xt_pool.tile([P, ND, SJ], QK_DT, tag="xtq", name=f"xk{j}")
                nc.sync.dma_start(
                    xk[:], kT_d.ap().rearrange("(o p) s -> p o s", p=P)[:, :, js])
                for g in range(2):
                    pk = pj_pool.tile([P, SJ], f32, tag="pj", name=f"pk{j}{g}")
                    for c in range(ND // 2):
                        nc.tensor.matmul(
                            pk[:], wk_sb[:, 2 * c:2 * c + 2, bass.ts(g, P)],
                            xk[:, 2 * c:2 * c + 2, :],
                            start=(c == 0), stop=(c == ND // 2 - 1),
                            perf_mode=DR)
                    nc.vector.tensor_copy(KTH[g][0][0:E, js], pk[0:E, :])
                    nc.vector.tensor_copy(
                        KTH[g][1][E:2 * E, js], pk[E:2 * E, :])

                xv = xt_pool.tile([P, ND, SJ], V_DT, tag="xtv", name=f"xv{j}")
                nc.sync.dma_start(
                    xv[:], vT_d.ap().rearrange("(o p) s -> p o s", p=P)[:, :, js])
                for u in range(SJ // P):
                    t = 4 * j + u
                    pv = pj_pool.tile([P, 2 * P], f32, tag="pj",
                                      name=f"pv{j}{u}")
                    for c in range(ND):
                        nc.tensor.matmul(
                            pv[:], xv[:, c, bass.ts(u, P)], wv_sb[:, c, :],
                            start=(c == 0), stop=(c == ND - 1))
                    for g in range(2):
                        nc.vector.tensor_copy(
                            V[g][:, t, :, 0:E],
                            pv[:, bass.ts(g, P)].rearrange(
                                "p (h e) -> p h e", h=2))

                # output projection for s-tile j-1, emitted BEFORE
                # attention(j) so it isn't stranded behind attention(j) in
                # the in-order PE stream (halves the exposed Wo tail); its
                # epilogue chain completed during this tile's projections.
                if j > 0:
                    emit_wo(j - 1)

                # attention for both head pairs on this s-tile
                for g in range(2):
                    nblk = 4 * j + 4
                    atp = [
                        at_pool.tile([P, SJ], f32, tag="at", name=f"at{g}{j}{h2}")
                        for h2 in range(2)
                    ]
                    for cb in range(nblk):
                        col0 = max(0, cb - 4 * j) * P
                        # both heads' score matmuls back to back: fp8 in,
                        # K=128 with the unused half zero (no switching)
                        scps = []
                        for h2 in range(2):
                            scp = sc_pool.tile(
                                [P, SJ], f32, tag="sc", name=f"sc{g}{j}{cb}{h2}")
                            nc.tensor.matmul(
                                scp[:, col0:],
                                KTH[g][h2][:, bass.ts(cb, P)],
                                QT[g][:, j * SJ + col0:(j + 1) * SJ],
                                start=True, stop=True)
                            scps.append(scp)
                        for h2 in range(2):
                            scp = scps[h2]
                            ex = ex_pool.tile(
                                [P, SJ], V_DT, tag="ex", name=f"ex{g}{j}{cb}{h2}")
                            nc.scalar.activation(
                                ex[:, col0:], scp[:, col0:], EXP, scale=1.0 / 32.0)
                            if cb >= 4 * j:
                                nc.vector.tensor_tensor(
                                    ex[:, col0:col0 + P], ex[:, col0:col0 + P],
                                    tri_sb[:], MULT)
                            nc.tensor.matmul(
                                atp[h2][0:E + 1, col0:],
                                V[g][:, cb, h2, 0:E + 1],
                                ex[:, col0:],
                                start=(cb == 0), stop=(cb == nblk - 1))
                    # epilogue: normalize by softmax denominator (row E).
                    # DMA moves the denominator row from PSUM partition E to
                    # partition 0 (DVE can't cross partitions); the approx
                    # reciprocal (~18 bits) runs ~5x faster than the exact
                    # one on this single-lane [1, SJ] tile.
                    for h2 in range(2):
                        rec = sm_pool.tile([1, SJ], f32, tag="rec",
                                           name=f"rec{g}{j}{h2}")
                        nc.vector.tensor_copy(rec[:], atp[h2][E:E + 1, :])
                        rec2 = sm_pool.tile([1, SJ], f32, tag="rec2",
                                            name=f"rec2{g}{j}{h2}")
                        nc.vector.reciprocal_approx_fast(out=rec2[:], in_=rec[:])
                        recb = sm_pool.tile([E, SJ], f32, tag="recb",
                                            name=f"recb{g}{j}{h2}")
                        nc.gpsimd.partition_broadcast(recb[:], rec2[:])
                        nc.vector.tensor_tensor(
                            attnG[g][bass.ts(h2, E), js], atp[h2][0:E, :],
                            recb[:], MULT)


            # tail: output projection for the last s-tile
            emit_wo(NJ - 1)

    nc.compile()
    return nc


def _get_nc():
    if not _NC_CACHE:
        _NC_CACHE.append(_build())
    return _NC_CACHE[0]


def _in_maps(q, k, v, W_q, W_k, W_v, W_o):
    qk_np = _NP_OF[QK_DT]
    v_np = _NP_OF[V_DT]
    tri = (np.arange(P)[:, None] <= np.arange(P)[None, :]).astype(v_np)
    xT = {}
    for b in range(B):
        xT[b] = (
            np.ascontiguousarray(q[b].T).astype(qk_np),
            np.ascontiguousarray(k[b].T).astype(qk_np),
            np.ascontiguousarray(v[b].T).astype(v_np),
        )
    maps = []
    for core in range(NCORES):
        b, quad = divmod(core, 4)
        hs = slice(4 * quad, 4 * quad + 4)
        qT_b, kT_b, vT_b = xT[b]
        maps.append({
            "qT": qT_b,
            "kT": kT_b,
            "vT": vT_b,
            # [4, D, E] -> [D, 4, E] -> [D, 256], col l*64+e = W[4q+l, d, e]
            "wq": np.ascontiguousarray(
                W_q[hs].transpose(1, 0, 2).reshape(D, 4 * E)).astype(qk_np),
            "wk": np.ascontiguousarray(
                W_k[hs].transpose(1, 0, 2).reshape(D, 4 * E)).astype(qk_np),
            "wv": np.ascontiguousarray(
                W_v[hs].transpose(1, 0, 2).reshape(D, 4 * E)).astype(v_np),
            # W_o[out, in] -> W_o.T rows for this quad's 256 input dims
            "wot": np.ascontiguousarray(
                W_o[:, 4 * quad * E:4 * quad * E + 4 * E].T).astype(
                    _NP_OF[O_DT]),
            "tri": tri,
        })
    return maps


def kernel(q, k, v, W_q, W_k, W_v, W_o, _trace=False, _trace_kwargs=None):
    q = np.asarray(q, dtype=np.float32)
    k = np.asarray(k, dtype=np.float32)
    v = np.asarray(v, dtype=np.float32)
    W_q = np.asarray(W_q, dtype=np.float32)
    W_k = np.asarray(W_k, dtype=np.float32)
    W_v = np.asarray(W_v, dtype=np.float32)
    W_o = np.asarray(W_o, dtype=np.float32)

    nc = _get_nc()
    maps = _in_maps(q, k, v, W_q, W_k, W_v, W_o)
    kwargs = dict(_trace_kwargs or {})
    res = run_bass_kernel_spmd(
        nc, maps, core_ids=list(range(NCORES)), trace=_trace, **kwargs)
    out = np.zeros((B, S, D), dtype=np.float32)
    for core in range(NCORES):
        b = core // 4
        out[b] += res.results[core]["out"]
    if _trace:
        kernel.last_results = res
    return out



# revision 34
# speedup vs baseline: 1.0126x; 1.0126x over previous
"""Causal multi-head attention (nn_Attention_87840671138123) on 8 trn2 NeuronCores.

Problem (B=2, S=2048, D=1024, H=16 heads, E=64 head_dim), fp32:
    Q = einsum('bsd,hde->bhse', q, W_q)   (same for K, V)
    scores = Q @ K^T / sqrt(D), causal mask, softmax
    attn = probs @ V  -> [B, S, D] (head-major concat)
    out = attn @ W_o.T

Sharding: core = 4*b + quad. Each core handles batch b and a quad of 4 heads
(heads 4*quad .. 4*quad+3). It computes a partial output
    out_part = attn_quad @ W_o.T[quad rows, :]   [S, D]
and the host sums the 4 partials per batch (the "all-reduce" of the output
projection done host-side at gather time).

Device layout choices (per core):
 - Host passes xT = x[b].T  [D, S] so the d-contraction sits on partitions.
 - Projections produce QT/KT in "transposed" layout [head-pair x 64, S]
   (head h2 of a pair occupies partitions 64*h2..64*h2+63), and V in natural
   [t, e] layout augmented with a ones-column (V_aug [t, 65]) so the
   attn matmul also accumulates the softmax denominator as row 64.
 - scoresT[t, s] = (KT chunk).T @ QT  -> exp on ACT (scale 1/32 folded in)
   -> causal handled by (a) skipping fully-masked blocks, (b) shrinking the
   moving dim to the valid s-range for diagonal blocks, (c) one [128,128]
   triangular mask multiply for the diagonal 128-col strip.
 - attnT_aug[65, s] += V_aug.T @ expT accumulated over t chunks in PSUM.
 - Normalize: denom row -> reciprocal -> partition_broadcast -> multiply.
 - Output projection: out[s, :] = sum_g (attnT chunk).T @ W_o.T slice.

Numerics: the Q/K path (inputs, weights, DoubleRow projections, stored
QT/KT, scores matmul) runs in fp8e4m3 — score errors are absolute-small
and only perturb softmax weights (~5e-3 relative on the output, vs the
2e-2 gate). The V path (V projection, attn*V) stays fp16 and the output
projection (attnG/WoT) bf16: value errors there pass straight through.

Performance model (measured): the NC clock is power/activity-throttled
(type-1 HAM throttle, half clock); MAC switching energy drives it, so
every padding element is zero and the hot paths use the narrowest dtypes
accuracy allows. fp8 scores + zero padding took the throttle duty from
~30% to ~14% of the run. Q/K projections use fp8 DoubleRow (K=256/matmul,
~2x). The epilogue normalize uses cross-base-partition DVE ops (PSUM row
64 -> partition 0, rows 0..63 -> 64..127) instead of SBUF-SBUF DMAs.
Measured on 8 cores: 229 us (staged baseline) -> 175.3 us.
"""

import ml_dtypes
import numpy as np

import concourse.bass as bass
import concourse.tile as tile
from concourse import bacc, mybir
from concourse.bass_utils import run_bass_kernel_spmd

B, S, D, H, E = 2, 2048, 1024, 16, 64
P = 128
NCORES = 8
SJ = 512            # s-tile width
NJ = S // SJ        # 4 s-tiles
ND = D // P         # 8 d-chunks
NT = S // P         # 16 t-chunks
f32 = mybir.dt.float32
f32r = mybir.dt.float32r
bf16 = mybir.dt.bfloat16
fp16 = mybir.dt.float16
EXP = mybir.ActivationFunctionType.Exp
MULT = mybir.AluOpType.mult

QK_DT = mybir.dt.float8e4   # q/k inputs, Wq/Wk, DoubleRow projection matmuls
SC_DT = mybir.dt.float8e4   # dtype of stored QT/KT + scores matmul (fp8e4m3)
V_DT = fp16         # dtype of v input, Wv, V_aug, expT
O_DT = bf16         # dtype of attnG / WoT (output projection matmul)
DR = mybir.MatmulPerfMode.DoubleRow

_NP_OF = {bf16: ml_dtypes.bfloat16, fp16: np.float16, f32r: np.float32,
          f32: np.float32, mybir.dt.float8e4: ml_dtypes.float8_e4m3}

_NC_CACHE = []


def _patch_ldw_opt():
    """Enable walrus LDWEIGHTS optimization (fast weight load).

    bass_utils.bir_verify_and_optimise hardcodes --enable-ldw-opt=false;
    wrap it to rewrite the flag. With ldw-opt on, walrus rejects standalone
    InstLdweights (CoreV3GenImpl.cpp:694) and instead generates the weight
    load itself from self-loading Matmults — so also rewrite bir.json to
    re-fuse each Ldweights+Matmult pair (bass tile_legalize splits them):
    drop the Ldweights, set ldweights=true on the Matmult, and merge the
    semaphore waits. A matmul carries one wait; extras (36 of 640 pairs)
    are parked on an inserted PE EventSemaphore just before the matmul.
    Verified numerically by the test harness.
    """
    import json as _json
    import os as _os

    from concourse import bass_utils as _bu
    if getattr(_bu, "_ldw_patched", False):
        return
    _orig_run = _bu.run_command

    def _rewrite_bir(path):
        with open(path) as f:
            d = _json.load(f)
        nfix = 0
        for fn in d.get("functions", []):
            for blk in fn.get("blocks", []):
                out = []
                pending = None
                for ins in blk["instructions"]:
                    eng = ins.get("engine")
                    op = ins.get("opcode")
                    if eng == "PE" and op == "Ldweights":
                        assert pending is None, "unpaired Ldweights"
                        pending = ins
                        continue
                    if eng == "PE" and op == "Matmult" and pending is not None:
                        waits = list(
                            (pending.get("sync_info") or {}).get("on_wait") or [])
                        si = ins.get("sync_info") or {
                            "on_update": [], "on_wait": []}
                        waits += list(si.get("on_wait") or [])
                        for w in waits[:-1]:
                            nfix += 1
                            out.append({
                                "debug": ins.get("debug", 0),
                                "engine": "PE",
                                "ins": [],
                                "name": f"I-ldwfix-{nfix}",
                                "opcode": "EventSemaphore",
                                "outs": [],
                                "sync_info": {"on_update": [], "on_wait": [w]},
                            })
                        si["on_wait"] = waits[-1:]
                        ins["sync_info"] = si
                        ins["ldweights"] = True
                        pending = None
                    out.append(ins)
                assert pending is None, "trailing Ldweights"
                blk["instructions"] = out
        with open(path, "w") as f:
            _json.dump(d, f)

    def _run(argv, **kw):
        argv = list(argv)
        if "--enable-ldw-opt=false" in argv:
            argv = ["--enable-ldw-opt=true" if a == "--enable-ldw-opt=false"
                    else a for a in argv]
            try:
                birp = argv[argv.index("-i") + 1]
            except ValueError:
                birp = None
            if birp is not None:
                cwd = kw.get("cwd")
                if cwd and not _os.path.isabs(birp):
                    birp = _os.path.join(cwd, birp)
                _rewrite_bir(birp)
        return _orig_run(argv, **kw)

    _bu.run_command = _run
    _bu._ldw_patched = True


def _build():
    _patch_ldw_opt()
    nc = bacc.Bacc("TRN2", target_bir_lowering=False, debug=False)

    qT_d = nc.dram_tensor("qT", [D, S], QK_DT, kind="ExternalInput")
    kT_d = nc.dram_tensor("kT", [D, S], QK_DT, kind="ExternalInput")
    vT_d = nc.dram_tensor("vT", [D, S], V_DT, kind="ExternalInput")
    wq_d = nc.dram_tensor("wq", [D, 4 * E], QK_DT, kind="ExternalInput")
    wk_d = nc.dram_tensor("wk", [D, 4 * E], QK_DT, kind="ExternalInput")
    wv_d = nc.dram_tensor("wv", [D, 4 * E], V_DT, kind="ExternalInput")
    wot_d = nc.dram_tensor("wot", [4 * E, D], O_DT, kind="ExternalInput")
    tri_d = nc.dram_tensor("tri", [P, P], V_DT, kind="ExternalInput")
    out_d = nc.dram_tensor("out", [S, D], f32, kind="ExternalOutput")

    with tile.TileContext(nc) as tc:
        with (
            tc.tile_pool(name="pers", bufs=1) as pers,
            tc.tile_pool(name="xt", bufs=4) as xt_pool,
            tc.tile_pool(name="ex", bufs=6) as ex_pool,
            tc.tile_pool(name="sm", bufs=6) as sm_pool,
            tc.tile_pool(name="ot", bufs=2) as ot_pool,
            tc.tile_pool(name="pj", bufs=2, space="PSUM") as pj_pool,
            tc.tile_pool(name="sc", bufs=3, space="PSUM") as sc_pool,
            tc.tile_pool(name="at", bufs=3, space="PSUM") as at_pool,
        ):
            # ---- persistent weights / constants ----
            wq_sb = pers.tile([P, ND, 4 * E], QK_DT, name="wq_sb")
            wk_sb = pers.tile([P, ND, 4 * E], QK_DT, name="wk_sb")
            wv_sb = pers.tile([P, ND, 4 * E], V_DT, name="wv_sb")
            nc.sync.dma_start(wq_sb[:], wq_d.ap().rearrange("(o p) m -> p o m", p=P))
            nc.sync.dma_start(wk_sb[:], wk_d.ap().rearrange("(o p) m -> p o m", p=P))
            nc.sync.dma_start(wv_sb[:], wv_d.ap().rearrange("(o p) m -> p o m", p=P))
            wot_sb = pers.tile([P, 2, D], O_DT, name="wot_sb")
            nc.sync.dma_start(wot_sb[:], wot_d.ap().rearrange("(g p) n -> p g n", p=P))
            tri_sb = pers.tile([P, P], V_DT, name="tri_sb")
            nc.sync.dma_start(tri_sb[:], tri_d.ap())

            # ---- persistent activations ----
            # The NC clock is power-throttled (type-1 activity throttle,
            # ~30% of the run at half clock in the bf16 baseline), so the
            # Q/K score path runs in fp8e4m3 (quantized after a bf16
            # projection) and every padding element is ZERO: zero weights
            # don't switch the PE multipliers, and MAC switching energy is
            # what drives the throttle. Score quantization only perturbs
            # softmax weights (~1% relative), not the value path.
            QT = [pers.tile([P, S], SC_DT, name=f"QT{g}") for g in range(2)]
            # Per-head KT zero-padded to 128 partitions: rows 0..63 hold the
            # head's K^T, rows 64..127 are zeros, so the K=128 scores matmul
            # nulls out the other head's Q rows in the shared QT rhs.
            KTH = [[pers.tile([P, S], SC_DT, name=f"KT{g}{h2}") for h2 in range(2)]
                   for g in range(2)]
            # V_aug padded to 128 cols: [64 V | ones | 63 zeros] so the attn
            # matmul loads all 128 PE columns (M=128).
            V = [pers.tile([P, NT, 2, P], V_DT, name=f"V{g}") for g in range(2)]
            attnG = [pers.tile([P, S], O_DT, name=f"attnG{g}") for g in range(2)]
            for g in range(2):
                nc.vector.memset(KTH[g][0][E:2 * E, :], 0.0)
                nc.vector.memset(KTH[g][1][0:E, :], 0.0)
                # only cols 0..E are ever loaded (65-col stationary)
                nc.vector.memset(V[g][:, :, :, E:E + 1], 1.0)

            def emit_wo(j):
                # output projection for s-tile j: out[s, :] = attnG.T @ WoT
                for u in range(SJ // P):
                    si = 4 * j + u
                    ot = ot_pool.tile([P, D], f32, tag="ot", name=f"ot{si}")
                    # g outer so both no-halves reuse the same stationary
                    # attnG block back-to-back; walrus dedups the repeat
                    # LDWEIGHTS (2 loads per si instead of 4)
                    pos = [pj_pool.tile([P, SJ], f32, tag="pj",
                                        name=f"po{si}{no}") for no in range(2)]
                    for g in range(2):
                        for no in range(2):
                            nc.tensor.matmul(
                                pos[no][:], attnG[g][:, bass.ts(si, P)],
                                wot_sb[:, g, bass.ts(no, SJ)],
                                start=(g == 0), stop=(g == 1))
                    for no in range(2):
                        nc.vector.tensor_copy(
                            ot[:, bass.ts(no, SJ)], pos[no][:])
                    nc.sync.dma_start(out_d.ap()[bass.ts(si, P), :], ot[:])

            # ---- fused per-s-tile pipeline: projections -> attention -> output ----
            for j in range(NJ):
                js = slice(j * SJ, (j + 1) * SJ)
                # Q/K projections in fp8 DoubleRow: each matmul contracts
                # K=256 (two 128-row k-tiles packed 2/cell), halving the PE
                # row count of the Q/K projections and the MAC energy.
                xq = xt_pool.tile([P, ND, SJ], QK_DT, tag="xtq", name=f"xq{j}")
                nc.sync.dma_start(
                    xq[:], qT_d.ap().rearrange("(o p) s -> p o s", p=P)[:, :, js])
                for g in range(2):
                    pq = pj_pool.tile([P, SJ], f32, tag="pj", name=f"pq{j}{g}")
                    for c in range(ND // 2):
                        nc.tensor.matmul(
                            pq[:], wq_sb[:, 2 * c:2 * c + 2, bass.ts(g, P)],
                            xq[:, 2 * c:2 * c + 2, :],
                            start=(c == 0), stop=(c == ND // 2 - 1),
                            perf_mode=DR)
                    nc.vector.tensor_copy(QT[g][:, js], pq[:])

                xk = xt_pool.tile([P, ND, SJ], QK_DT, tag="xtq", name=f"xk{j}")
                nc.sync.dma_start(
                    xk[:], kT_d.ap().rearrange("(o p) s -> p o s", p=P)[:, :, js])
                for g in range(2):
                    pk = pj_pool.tile([P, SJ], f32, tag="pj", name=f"pk{j}{g}")
                    for c in range(ND // 2):
                        nc.tensor.matmul(
                            pk[:], wk_sb[:, 2 * c:2 * c + 2, bass.ts(g, P)],
                            xk[:, 2 * c:2 * c + 2, :],
                            start=(c == 0), stop=(c == ND // 2 - 1),
                            perf_mode=DR)
                    nc.vector.tensor_copy(KTH[g][0][0:E, js], pk[0:E, :])
                    nc.vector.tensor_copy(
                        KTH[g][1][E:2 * E, js], pk[E:2 * E, :])

                xv = xt_pool.tile([P, ND, SJ], V_DT, tag="xtv", name=f"xv{j}")
                nc.sync.dma_start(
                    xv[:], vT_d.ap().rearrange("(o p) s -> p o s", p=P)[:, :, js])
                for u in range(SJ // P):
                    t = 4 * j + u
                    pv = pj_pool.tile([P, 2 * P], f32, tag="pj",
                                      name=f"pv{j}{u}")
                    for c in range(ND):
                        nc.tensor.matmul(
                            pv[:], xv[:, c, bass.ts(u, P)], wv_sb[:, c, :],
                            start=(c == 0), stop=(c == ND - 1))
                    for g in range(2):
                        nc.vector.tensor_copy(
                            V[g][:, t, :, 0:E],
                            pv[:, bass.ts(g, P)].rearrange(
                                "p (h e) -> p h e", h=2))

                # output projection for s-tile j-1, emitted BEFORE
                # attention(j) so it isn't stranded behind attention(j) in
                # the in-order PE stream (halves the exposed Wo tail); its
                # epilogue chain completed during this tile's projections.
                if j > 0:
                    emit_wo(j - 1)

                # attention for both head pairs on this s-tile
                for g in range(2):
                    nblk = 4 * j + 4
                    atp = [
                        at_pool.tile([P, SJ], f32, tag="at", name=f"at{g}{j}{h2}")
                        for h2 in range(2)
                    ]
                    for cb in range(nblk):
                        col0 = max(0, cb - 4 * j) * P
                        # both heads' score matmuls back to back: fp8 in,
                        # K=128 with the unused half zero (no switching)
                        scps = []
                        for h2 in range(2):
                            scp = sc_pool.tile(
                                [P, SJ], f32, tag="sc", name=f"sc{g}{j}{cb}{h2}")
                            nc.tensor.matmul(
                                scp[:, col0:],
                                KTH[g][h2][:, bass.ts(cb, P)],
                                QT[g][:, j * SJ + col0:(j + 1) * SJ],
                                start=True, stop=True)
                            scps.append(scp)
                        for h2 in range(2):
                            scp = scps[h2]
                            ex = ex_pool.tile(
                                [P, SJ], V_DT, tag="ex", name=f"ex{g}{j}{cb}{h2}")
                            nc.scalar.activation(
                                ex[:, col0:], scp[:, col0:], EXP, scale=1.0 / 32.0)
                            if cb >= 4 * j:
                                nc.vector.tensor_tensor(
                                    ex[:, col0:col0 + P], ex[:, col0:col0 + P],
                                    tri_sb[:], MULT)
                            nc.tensor.matmul(
                                atp[h2][0:E + 1, col0:],
                                V[g][:, cb, h2, 0:E + 1],
                                ex[:, col0:],
                                start=(cb == 0), stop=(cb == nblk - 1))
                    # epilogue: normalize by softmax denominator (row E).
                    # DMA moves the denominator row from PSUM partition E to
                    # partition 0 (DVE can't cross partitions); the approx
                    # reciprocal (~18 bits) runs ~5x faster than the exact
                    # one on this single-lane [1, SJ] tile.
                    for h2 in range(2):
                        rec = sm_pool.tile([1, SJ], f32, tag="rec",
                                           name=f"rec{g}{j}{h2}")
                        nc.vector.tensor_copy(rec[:], atp[h2][E:E + 1, :])
                        rec2 = sm_pool.tile([1, SJ], f32, tag="rec2",
                                            name=f"rec2{g}{j}{h2}")
                        nc.vector.reciprocal_approx_fast(out=rec2[:], in_=rec[:])
                        recb = sm_pool.tile([E, SJ], f32, tag="recb",
                                            name=f"recb{g}{j}{h2}")
                        nc.gpsimd.partition_broadcast(recb[:], rec2[:])
                        nc.vector.tensor_tensor(
                            attnG[g][bass.ts(h2, E), js], atp[h2][0:E, :],
                            recb[:], MULT)


            # tail: output projection for the last s-tile
            emit_wo(NJ - 1)

    nc.compile()
    return nc


def _get_nc():
    if not _NC_CACHE:
        _NC_CACHE.append(_build())
    return _NC_CACHE[0]


def _in_maps(q, k, v, W_q, W_k, W_v, W_o):
    qk_np = _NP_OF[QK_DT]
    v_np = _NP_OF[V_DT]
    tri = (np.arange(P)[:, None] <= np.arange(P)[None, :]).astype(v_np)
    xT = {}
    for b in range(B):
        xT[b] = (
            np.ascontiguousarray(q[b].T).astype(qk_np),
            np.ascontiguousarray(k[b].T).astype(qk_np),
            np.ascontiguousarray(v[b].T).astype(v_np),
        )
    maps = []
    for core in range(NCORES):
        b, quad = divmod(core, 4)
        hs = slice(4 * quad, 4 * quad + 4)
        qT_b, kT_b, vT_b = xT[b]
        maps.append({
            "qT": qT_b,
            "kT": kT_b,
            "vT": vT_b,
            # [4, D, E] -> [D, 4, E] -> [D, 256], col l*64+e = W[4q+l, d, e]
            "wq": np.ascontiguousarray(
                W_q[hs].transpose(1, 0, 2).reshape(D, 4 * E)).astype(qk_np),
            "wk": np.ascontiguousarray(
                W_k[hs].transpose(1, 0, 2).reshape(D, 4 * E)).astype(qk_np),
            "wv": np.ascontiguousarray(
                W_v[hs].transpose(1, 0, 2).reshape(D, 4 * E)).astype(v_np),
            # W_o[out, in] -> W_o.T rows for this quad's 256 input dims
            "wot": np.ascontiguousarray(
                W_o[:, 4 * quad * E:4 * quad * E + 4 * E].T).astype(
                    _NP_OF[O_DT]),
            "tri": tri,
        })
    return maps


def kernel(q, k, v, W_q, W_k, W_v, W_o, _trace=False, _trace_kwargs=None):
    q = np.asarray(q, dtype=np.float32)
    k = np.asarray(k, dtype=np.float32)
    v = np.asarray(v, dtype=np.float32)
    W_q = np.asarray(W_q, dtype=np.float32)
    W_k = np.asarray(W_k, dtype=np.float32)
    W_v = np.asarray(W_v, dtype=np.float32)
    W_o = np.asarray(W_o, dtype=np.float32)

    nc = _get_nc()
    maps = _in_maps(q, k, v, W_q, W_k, W_v, W_o)
    kwargs = dict(_trace_kwargs or {})
    res = run_bass_kernel_spmd(
        nc, maps, core_ids=list(range(NCORES)), trace=_trace, **kwargs)
    out = np.zeros((B, S, D), dtype=np.float32)
    for core in range(NCORES):
        b = core // 4
        out[b] += res.results[core]["out"]
    if _trace:
        kernel.last_results = res
    return out



# revision 35
# speedup vs baseline: 1.0135x; 1.0009x over previous
"""Causal multi-head attention (nn_Attention_87840671138123) on 8 trn2 NeuronCores.

Problem (B=2, S=2048, D=1024, H=16 heads, E=64 head_dim), fp32:
    Q = einsum('bsd,hde->bhse', q, W_q)   (same for K, V)
    scores = Q @ K^T / sqrt(D), causal mask, softmax
    attn = probs @ V  -> [B, S, D] (head-major concat)
    out = attn @ W_o.T

Sharding: core = 4*b + quad. Each core handles batch b and a quad of 4 heads
(heads 4*quad .. 4*quad+3). It computes a partial output
    out_part = attn_quad @ W_o.T[quad rows, :]   [S, D]
and the host sums the 4 partials per batch (the "all-reduce" of the output
projection done host-side at gather time).

Device layout choices (per core):
 - Host passes xT = x[b].T  [D, S] so the d-contraction sits on partitions.
 - Projections produce QT/KT in "transposed" layout [head-pair x 64, S]
   (head h2 of a pair occupies partitions 64*h2..64*h2+63), and V in natural
   [t, e] layout augmented with a ones-column (V_aug [t, 65]) so the
   attn matmul also accumulates the softmax denominator as row 64.
 - scoresT[t, s] = (KT chunk).T @ QT  -> exp on ACT (scale 1/32 folded in)
   -> causal handled by (a) skipping fully-masked blocks, (b) shrinking the
   moving dim to the valid s-range for diagonal blocks, (c) one [128,128]
   triangular mask multiply for the diagonal 128-col strip.
 - attnT_aug[65, s] += V_aug.T @ expT accumulated over t chunks in PSUM.
 - Normalize: denom row -> reciprocal -> partition_broadcast -> multiply.
 - Output projection: out[s, :] = sum_g (attnT chunk).T @ W_o.T slice.

Numerics: the Q/K path (inputs, weights, DoubleRow projections, stored
QT/KT, scores matmul) runs in fp8e4m3 — score errors are absolute-small
and only perturb softmax weights (~5e-3 relative on the output, vs the
2e-2 gate). The V path (V projection, attn*V) stays fp16 and the output
projection (attnG/WoT) bf16: value errors there pass straight through.

Performance model (measured): the NC clock is power/activity-throttled
(type-1 HAM throttle, half clock); MAC switching energy drives it, so
every padding element is zero and the hot paths use the narrowest dtypes
accuracy allows. fp8 scores + zero padding took the throttle duty from
~30% to ~14% of the run. Q/K projections use fp8 DoubleRow (K=256/matmul,
~2x). The epilogue normalize uses cross-base-partition DVE ops (PSUM row
64 -> partition 0, rows 0..63 -> 64..127) instead of SBUF-SBUF DMAs.
Measured on 8 cores: 229 us (staged baseline) -> 175.3 us.
"""

import ml_dtypes
import numpy as np

import concourse.bass as bass
import concourse.tile as tile
from concourse import bacc, mybir
from concourse.bass_utils import run_bass_kernel_spmd

B, S, D, H, E = 2, 2048, 1024, 16, 64
P = 128
NCORES = 8
SJ = 512            # s-tile width
NJ = S // SJ        # 4 s-tiles
ND = D // P         # 8 d-chunks
NT = S // P         # 16 t-chunks
f32 = mybir.dt.float32
f32r = mybir.dt.float32r
bf16 = mybir.dt.bfloat16
fp16 = mybir.dt.float16
EXP = mybir.ActivationFunctionType.Exp
MULT = mybir.AluOpType.mult

QK_DT = mybir.dt.float8e4   # q/k inputs, Wq/Wk, DoubleRow projection matmuls
SC_DT = mybir.dt.float8e4   # dtype of stored QT/KT + scores matmul (fp8e4m3)
V_DT = fp16         # dtype of v input, Wv, V_aug, expT
O_DT = bf16         # dtype of attnG / WoT (output projection matmul)
DR = mybir.MatmulPerfMode.DoubleRow

_NP_OF = {bf16: ml_dtypes.bfloat16, fp16: np.float16, f32r: np.float32,
          f32: np.float32, mybir.dt.float8e4: ml_dtypes.float8_e4m3}

_NC_CACHE = []


def _patch_ldw_opt():
    """Enable walrus LDWEIGHTS optimization (fast weight load).

    bass_utils.bir_verify_and_optimise hardcodes --enable-ldw-opt=false;
    wrap it to rewrite the flag. With ldw-opt on, walrus rejects standalone
    InstLdweights (CoreV3GenImpl.cpp:694) and instead generates the weight
    load itself from self-loading Matmults — so also rewrite bir.json to
    re-fuse each Ldweights+Matmult pair (bass tile_legalize splits them):
    drop the Ldweights, set ldweights=true on the Matmult, and merge the
    semaphore waits. A matmul carries one wait; extras (36 of 640 pairs)
    are parked on an inserted PE EventSemaphore just before the matmul.
    Verified numerically by the test harness.
    """
    import json as _json
    import os as _os

    from concourse import bass_utils as _bu
    if getattr(_bu, "_ldw_patched", False):
        return
    _orig_run = _bu.run_command

    def _rewrite_bir(path):
        with open(path) as f:
            d = _json.load(f)
        nfix = 0
        for fn in d.get("functions", []):
            for blk in fn.get("blocks", []):
                out = []
                pending = None
                for ins in blk["instructions"]:
                    eng = ins.get("engine")
                    op = ins.get("opcode")
                    if eng == "PE" and op == "Ldweights":
                        assert pending is None, "unpaired Ldweights"
                        pending = ins
                        continue
                    if eng == "PE" and op == "Matmult" and pending is not None:
                        waits = list(
                            (pending.get("sync_info") or {}).get("on_wait") or [])
                        si = ins.get("sync_info") or {
                            "on_update": [], "on_wait": []}
                        waits += list(si.get("on_wait") or [])
                        for w in waits[:-1]:
                            nfix += 1
                            out.append({
                                "debug": ins.get("debug", 0),
                                "engine": "PE",
                                "ins": [],
                                "name": f"I-ldwfix-{nfix}",
                                "opcode": "EventSemaphore",
                                "outs": [],
                                "sync_info": {"on_update": [], "on_wait": [w]},
                            })
                        si["on_wait"] = waits[-1:]
                        ins["sync_info"] = si
                        ins["ldweights"] = True
                        pending = None
                    out.append(ins)
                assert pending is None, "trailing Ldweights"
                blk["instructions"] = out
        with open(path, "w") as f:
            _json.dump(d, f)

    def _run(argv, **kw):
        argv = list(argv)
        if "--enable-ldw-opt=false" in argv:
            argv = ["--enable-ldw-opt=true" if a == "--enable-ldw-opt=false"
                    else a for a in argv]
            try:
                birp = argv[argv.index("-i") + 1]
            except ValueError:
                birp = None
            if birp is not None:
                cwd = kw.get("cwd")
                if cwd and not _os.path.isabs(birp):
                    birp = _os.path.join(cwd, birp)
                _rewrite_bir(birp)
        return _orig_run(argv, **kw)

    _bu.run_command = _run
    _bu._ldw_patched = True


def _build():
    _patch_ldw_opt()
    nc = bacc.Bacc("TRN2", target_bir_lowering=False, debug=False)

    qT_d = nc.dram_tensor("qT", [D, S], QK_DT, kind="ExternalInput")
    kT_d = nc.dram_tensor("kT", [D, S], QK_DT, kind="ExternalInput")
    vT_d = nc.dram_tensor("vT", [D, S], V_DT, kind="ExternalInput")
    wq_d = nc.dram_tensor("wq", [D, 4 * E], QK_DT, kind="ExternalInput")
    wk_d = nc.dram_tensor("wk", [D, 4 * E], QK_DT, kind="ExternalInput")
    wv_d = nc.dram_tensor("wv", [D, 4 * E], V_DT, kind="ExternalInput")
    wot_d = nc.dram_tensor("wot", [4 * E, D], O_DT, kind="ExternalInput")
    tri_d = nc.dram_tensor("tri", [P, P], V_DT, kind="ExternalInput")
    out_d = nc.dram_tensor("out", [S, D], f32, kind="ExternalOutput")

    with tile.TileContext(nc) as tc:
        with (
            tc.tile_pool(name="pers", bufs=1) as pers,
            tc.tile_pool(name="xt", bufs=6) as xt_pool,
            tc.tile_pool(name="ex", bufs=6) as ex_pool,
            tc.tile_pool(name="sm", bufs=8) as sm_pool,
            tc.tile_pool(name="ot", bufs=2) as ot_pool,
            tc.tile_pool(name="pj", bufs=2, space="PSUM") as pj_pool,
            tc.tile_pool(name="sc", bufs=3, space="PSUM") as sc_pool,
            tc.tile_pool(name="at", bufs=3, space="PSUM") as at_pool,
        ):
            # ---- persistent weights / constants ----
            wq_sb = pers.tile([P, ND, 4 * E], QK_DT, name="wq_sb")
            wk_sb = pers.tile([P, ND, 4 * E], QK_DT, name="wk_sb")
            wv_sb = pers.tile([P, ND, 4 * E], V_DT, name="wv_sb")
            nc.sync.dma_start(wq_sb[:], wq_d.ap().rearrange("(o p) m -> p o m", p=P))
            nc.sync.dma_start(wk_sb[:], wk_d.ap().rearrange("(o p) m -> p o m", p=P))
            nc.sync.dma_start(wv_sb[:], wv_d.ap().rearrange("(o p) m -> p o m", p=P))
            wot_sb = pers.tile([P, 2, D], O_DT, name="wot_sb")
            nc.sync.dma_start(wot_sb[:], wot_d.ap().rearrange("(g p) n -> p g n", p=P))
            tri_sb = pers.tile([P, P], V_DT, name="tri_sb")
            nc.sync.dma_start(tri_sb[:], tri_d.ap())

            # ---- persistent activations ----
            # The NC clock is power-throttled (type-1 activity throttle,
            # ~30% of the run at half clock in the bf16 baseline), so the
            # Q/K score path runs in fp8e4m3 (quantized after a bf16
            # projection) and every padding element is ZERO: zero weights
            # don't switch the PE multipliers, and MAC switching energy is
            # what drives the throttle. Score quantization only perturbs
            # softmax weights (~1% relative), not the value path.
            QT = [pers.tile([P, S], SC_DT, name=f"QT{g}") for g in range(2)]
            # Per-head KT zero-padded to 128 partitions: rows 0..63 hold the
            # head's K^T, rows 64..127 are zeros, so the K=128 scores matmul
            # nulls out the other head's Q rows in the shared QT rhs.
            KTH = [[pers.tile([P, S], SC_DT, name=f"KT{g}{h2}") for h2 in range(2)]
                   for g in range(2)]
            # V_aug padded to 128 cols: [64 V | ones | 63 zeros] so the attn
            # matmul loads all 128 PE columns (M=128).
            V = [pers.tile([P, NT, 2, P], V_DT, name=f"V{g}") for g in range(2)]
            attnG = [pers.tile([P, S], O_DT, name=f"attnG{g}") for g in range(2)]
            for g in range(2):
                nc.vector.memset(KTH[g][0][E:2 * E, :], 0.0)
                nc.vector.memset(KTH[g][1][0:E, :], 0.0)
                # only cols 0..E are ever loaded (65-col stationary)
                nc.vector.memset(V[g][:, :, :, E:E + 1], 1.0)

            def emit_wo(j):
                # output projection for s-tile j: out[s, :] = attnG.T @ WoT
                for u in range(SJ // P):
                    si = 4 * j + u
                    ot = ot_pool.tile([P, D], f32, tag="ot", name=f"ot{si}")
                    # g outer so both no-halves reuse the same stationary
                    # attnG block back-to-back; walrus dedups the repeat
                    # LDWEIGHTS (2 loads per si instead of 4)
                    pos = [pj_pool.tile([P, SJ], f32, tag="pj",
                                        name=f"po{si}{no}") for no in range(2)]
                    for g in range(2):
                        for no in range(2):
                            nc.tensor.matmul(
                                pos[no][:], attnG[g][:, bass.ts(si, P)],
                                wot_sb[:, g, bass.ts(no, SJ)],
                                start=(g == 0), stop=(g == 1))
                    for no in range(2):
                        nc.vector.tensor_copy(
                            ot[:, bass.ts(no, SJ)], pos[no][:])
                    nc.sync.dma_start(out_d.ap()[bass.ts(si, P), :], ot[:])

            # ---- fused per-s-tile pipeline: projections -> attention -> output ----
            for j in range(NJ):
                js = slice(j * SJ, (j + 1) * SJ)
                # Q/K projections in fp8 DoubleRow: each matmul contracts
                # K=256 (two 128-row k-tiles packed 2/cell), halving the PE
                # row count of the Q/K projections and the MAC energy.
                xq = xt_pool.tile([P, ND, SJ], QK_DT, tag="xtq", name=f"xq{j}")
                nc.sync.dma_start(
                    xq[:], qT_d.ap().rearrange("(o p) s -> p o s", p=P)[:, :, js])
                for g in range(2):
                    pq = pj_pool.tile([P, SJ], f32, tag="pj", name=f"pq{j}{g}")
                    for c in range(ND // 2):
                        nc.tensor.matmul(
                            pq[:], wq_sb[:, 2 * c:2 * c + 2, bass.ts(g, P)],
                            xq[:, 2 * c:2 * c + 2, :],
                            start=(c == 0), stop=(c == ND // 2 - 1),
                            perf_mode=DR)
                    nc.vector.tensor_copy(QT[g][:, js], pq[:])

                xk = xt_pool.tile([P, ND, SJ], QK_DT, tag="xtq", name=f"xk{j}")
                nc.sync.dma_start(
                    xk[:], kT_d.ap().rearrange("(o p) s -> p o s", p=P)[:, :, js])
                for g in range(2):
                    pk = pj_pool.tile([P, SJ], f32, tag="pj", name=f"pk{j}{g}")
                    for c in range(ND // 2):
                        nc.tensor.matmul(
                            pk[:], wk_sb[:, 2 * c:2 * c + 2, bass.ts(g, P)],
                            xk[:, 2 * c:2 * c + 2, :],
                            start=(c == 0), stop=(c == ND // 2 - 1),
                            perf_mode=DR)
                    nc.vector.tensor_copy(KTH[g][0][0:E, js], pk[0:E, :])
                    nc.vector.tensor_copy(
                        KTH[g][1][E:2 * E, js], pk[E:2 * E, :])

                xv = xt_pool.tile([P, ND, SJ], V_DT, tag="xtv", name=f"xv{j}")
                nc.sync.dma_start(
                    xv[:], vT_d.ap().rearrange("(o p) s -> p o s", p=P)[:, :, js])
                for u in range(SJ // P):
                    t = 4 * j + u
                    pv = pj_pool.tile([P, 2 * P], f32, tag="pj",
                                      name=f"pv{j}{u}")
                    for c in range(ND):
                        nc.tensor.matmul(
                            pv[:], xv[:, c, bass.ts(u, P)], wv_sb[:, c, :],
                            start=(c == 0), stop=(c == ND - 1))
                    for g in range(2):
                        nc.vector.tensor_copy(
                            V[g][:, t, :, 0:E],
                            pv[:, bass.ts(g, P)].rearrange(
                                "p (h e) -> p h e", h=2))

                # output projection for s-tile j-1, emitted BEFORE
                # attention(j) so it isn't stranded behind attention(j) in
                # the in-order PE stream (halves the exposed Wo tail); its
                # epilogue chain completed during this tile's projections.
                if j > 0:
                    emit_wo(j - 1)

                # attention for both head pairs on this s-tile
                for g in range(2):
                    nblk = 4 * j + 4
                    atp = [
                        at_pool.tile([P, SJ], f32, tag="at", name=f"at{g}{j}{h2}")
                        for h2 in range(2)
                    ]
                    for cb in range(nblk):
                        col0 = max(0, cb - 4 * j) * P
                        # both heads' score matmuls back to back: fp8 in,
                        # K=128 with the unused half zero (no switching)
                        scps = []
                        for h2 in range(2):
                            scp = sc_pool.tile(
                                [P, SJ], f32, tag="sc", name=f"sc{g}{j}{cb}{h2}")
                            nc.tensor.matmul(
                                scp[:, col0:],
                                KTH[g][h2][:, bass.ts(cb, P)],
                                QT[g][:, j * SJ + col0:(j + 1) * SJ],
                                start=True, stop=True)
                            scps.append(scp)
                        for h2 in range(2):
                            scp = scps[h2]
                            ex = ex_pool.tile(
                                [P, SJ], V_DT, tag="ex", name=f"ex{g}{j}{cb}{h2}")
                            nc.scalar.activation(
                                ex[:, col0:], scp[:, col0:], EXP, scale=1.0 / 32.0)
                            if cb >= 4 * j:
                                nc.vector.tensor_tensor(
                                    ex[:, col0:col0 + P], ex[:, col0:col0 + P],
                                    tri_sb[:], MULT)
                            nc.tensor.matmul(
                                atp[h2][0:E + 1, col0:],
                                V[g][:, cb, h2, 0:E + 1],
                                ex[:, col0:],
                                start=(cb == 0), stop=(cb == nblk - 1))
                    # epilogue: normalize by softmax denominator (row E).
                    # DMA moves the denominator row from PSUM partition E to
                    # partition 0 (DVE can't cross partitions); the approx
                    # reciprocal (~18 bits) runs ~5x faster than the exact
                    # one on this single-lane [1, SJ] tile.
                    for h2 in range(2):
                        rec = sm_pool.tile([1, SJ], f32, tag="rec",
                                           name=f"rec{g}{j}{h2}")
                        nc.vector.tensor_copy(rec[:], atp[h2][E:E + 1, :])
                        rec2 = sm_pool.tile([1, SJ], f32, tag="rec2",
                                            name=f"rec2{g}{j}{h2}")
                        nc.vector.reciprocal_approx_fast(out=rec2[:], in_=rec[:])
                        recb = sm_pool.tile([E, SJ], f32, tag="recb",
                                            name=f"recb{g}{j}{h2}")
                        nc.gpsimd.partition_broadcast(recb[:], rec2[:])
                        nc.vector.tensor_tensor(
                            attnG[g][bass.ts(h2, E), js], atp[h2][0:E, :],
                            recb[:], MULT)


            # tail: output projection for the last s-tile
            emit_wo(NJ - 1)

    nc.compile()
    return nc


def _get_nc():
    if not _NC_CACHE:
        _NC_CACHE.append(_build())
    return _NC_CACHE[0]


def _in_maps(q, k, v, W_q, W_k, W_v, W_o):
    qk_np = _NP_OF[QK_DT]
    v_np = _NP_OF[V_DT]
    tri = (np.arange(P)[:, None] <= np.arange(P)[None, :]).astype(v_np)
    xT = {}
    for b in range(B):
        xT[b] = (
            np.ascontiguousarray(q[b].T).astype(qk_np),
            np.ascontiguousarray(k[b].T).astype(qk_np),
            np.ascontiguousarray(v[b].T).astype(v_np),
        )
    maps = []
    for core in range(NCORES):
        b, quad = divmod(core, 4)
        hs = slice(4 * quad, 4 * quad + 4)
        qT_b, kT_b, vT_b = xT[b]
        maps.append({
            "qT": qT_b,
            "kT": kT_b,
            "vT": vT_b,
            # [4, D, E] -> [D, 4, E] -> [D, 256], col l*64+e = W[4q+l, d, e]
            "wq": np.ascontiguousarray(
                W_q[hs].transpose(1, 0, 2).reshape(D, 4 * E)).astype(qk_np),
            "wk": np.ascontiguousarray(
                W_k[hs].transpose(1, 0, 2).reshape(D, 4 * E)).astype(qk_np),
            "wv": np.ascontiguousarray(
                W_v[hs].transpose(1, 0, 2).reshape(D, 4 * E)).astype(v_np),
            # W_o[out, in] -> W_o.T rows for this quad's 256 input dims
            "wot": np.ascontiguousarray(
                W_o[:, 4 * quad * E:4 * quad * E + 4 * E].T).astype(
                    _NP_OF[O_DT]),
            "tri": tri,
        })
    return maps


def kernel(q, k, v, W_q, W_k, W_v, W_o, _trace=False, _trace_kwargs=None):
    q = np.asarray(q, dtype=np.float32)
    k = np.asarray(k, dtype=np.float32)
    v = np.asarray(v, dtype=np.float32)
    W_q = np.asarray(W_q, dtype=np.float32)
    W_k = np.asarray(W_k, dtype=np.float32)
    W_v = np.asarray(W_v, dtype=np.float32)
    W_o = np.asarray(W_o, dtype=np.float32)

    nc = _get_nc()
    maps = _in_maps(q, k, v, W_q, W_k, W_v, W_o)
    kwargs = dict(_trace_kwargs or {})
    res = run_bass_kernel_spmd(
        nc, maps, core_ids=list(range(NCORES)), trace=_trace, **kwargs)
    out = np.zeros((B, S, D), dtype=np.float32)
    for core in range(NCORES):
        b = core // 4
        out[b] += res.results[core]["out"]
    if _trace:
        kernel.last_results = res
    return out

